# revision 26
# baseline (speedup 1.0000x reference)
"""Trainium2 Bass kernel for nn_DotProductionAttention.

Per batch b (8 batches, one NeuronCore each — data parallel):
    S    = output[b] @ context[b].T + mask   [1024, 2048]   (fp32r matmuls)
    P    = softmax(S, axis=-1)               -> attn output (f32)
    mix  = P @ context[b]                    [1024, 1024]   (bf16 matmuls)
    out  = tanh(concat(mix, output[b]) @ W.T + b)           (bf16 matmuls)

Host pre-processing (free): Q^T/K^T/W^T transposes, bf16 casts, bool mask ->
additive f32 row replicated to 128 partitions, bias replicated to 128 rows.

kernel(**inputs) takes FULL inputs, shards batch across 8 cores, returns
(out, attn) like the reference.
"""

import numpy as np
import ml_dtypes

import concourse.bass as bass
import concourse.mybir as mybir
import concourse.tile as tile
from concourse import bacc, bass_utils
from concourse.masks import make_identity

F32 = mybir.dt.float32
F32R = mybir.dt.float32r
F16 = mybir.dt.float16
BF16 = mybir.dt.bfloat16

B, LQ, LK, D = 8, 1024, 2048, 1024
N_CORES = 8
NEG_BIG = np.float32(-1.0e30)

NDQ = LQ // 128    # 8 q-tiles
NDC = D // 128     # 8 contraction chunks over D
NKC = LK // 512    # 4 k-chunks of 512
NKP = LK // 128    # 16 k-partition chunks
NEC = 2 * D // 128  # 16 e-chunks for the linear layer


def _build_program(repeat=1, phases=(1, 2, 3), dt1=F16, ablate=()):
    """Build the per-core Bass program (same NEFF on all 8 cores)."""
    nc = bacc.Bacc("TRN2", target_bir_lowering=False, debug=False)

    # ---- DRAM I/O (per core = one batch) ----
    qT_r = nc.dram_tensor("qT_r", [D, LQ], dt1, kind="ExternalInput").ap()
    kT_r = nc.dram_tensor("kT_r", [D, LK], dt1, kind="ExternalInput").ap()
    v_b = nc.dram_tensor("v_b", [LK, D], F16, kind="ExternalInput").ap()
    wT_b = nc.dram_tensor("wT_b", [2 * D, D], F16, kind="ExternalInput").ap()
    maskb = nc.dram_tensor("maskb", [128, LK], BF16, kind="ExternalInput").ap()
    bias_b = nc.dram_tensor("bias_b", [128, D], F32, kind="ExternalInput").ap()
    attn_o = nc.dram_tensor("attn_o", [LQ, LK], F32, kind="ExternalOutput").ap()
    out_o = nc.dram_tensor("out_o", [LQ, D], F32, kind="ExternalOutput").ap()

    qT_r3 = qT_r.rearrange("(c p) q -> p c q", p=128)
    kT_r3 = kT_r.rearrange("(c p) k -> p c k", p=128)
    v_b3 = v_b.rearrange("(c p) d -> p c d", p=128)
    wT_b3 = wT_b.rearrange("(c p) d -> p c d", p=128)

    with tile.TileContext(nc) as tc:
        const_pool = tc.alloc_tile_pool(name="const_pool", bufs=1)
        t_ident = const_pool.tile([128, 128], F16)
        make_identity(nc, t_ident[:])

        # persistent phases 1-2: P^T bf16 [k, q] as [128, 16, 1024]
        pt_pool = tc.alloc_tile_pool(name="pt_pool", bufs=1)
        t_PT = pt_pool.tile([128, NKP, LQ], F16)

        # V resident from the start: DMAs emitted mid-phase-1 so the
        # transfers hide behind phase-1 compute and phase 2 starts hot.
        v_pool = tc.alloc_tile_pool(name="v_pool", bufs=1)
        t_v = v_pool.tile([128, NKP, D], F16)

        # Q^T bf16 for phase 3, cast on-chip from the streamed fp16 q-tiles
        qtb_pool = tc.alloc_tile_pool(name="qtb_pool", bufs=1)
        t_qTb = qtb_pool.tile([128, NDC, LQ], F16)

        def emit_body():
            # ---------------- Phase 1: S, softmax, P, P^T ----------------
            with (
                tc.tile_pool(name="p1_big", bufs=1) as p1_big,
                tc.tile_pool(name="p1_qt", bufs=3) as p1_qt,
                tc.tile_pool(name="p1_work", bufs=1) as p1_work,
                tc.tile_pool(name="p1_p", bufs=1) as p1_p,
                tc.tile_pool(name="p1_pb", bufs=2) as p1_pb,
                tc.tile_pool(name="ps_s", bufs=6, space="PSUM") as ps_s,
                tc.tile_pool(name="ps_t", bufs=2, space="PSUM") as ps_t,
            ):
                t_kT = p1_big.tile([128, NDC, LK], dt1)
                t_mask = p1_big.tile([128, LK], BF16)
                # DMA issue order tuned for the serial HBM pipe: the first
                # q-tile and K^T chunk lead, everything else interleaves.
                qt_tiles = {}

                def load_qt(qt):
                    t = p1_qt.tile([128, NDC, 128], dt1, tag="t_qT",
                                   name=f"t_qT_{qt}")
                    nc.sync.dma_start(t[:], qT_r3[:, :, qt * 128 : (qt + 1) * 128])
                    qt_tiles[qt] = t

                def load_kt(kc):
                    nc.sync.dma_start(
                        t_kT[:, :, kc * 512 : (kc + 1) * 512],
                        kT_r3[:, :, kc * 512 : (kc + 1) * 512],
                    )

                load_qt(0)
                load_kt(0)
                load_qt(1)
                nc.sync.dma_start(t_mask[:], maskb[:])
                load_kt(1)
                load_qt(2)
                load_kt(2)
                load_kt(3)

                prev_pb = None  # software-pipelined transpose input

                def emit_transposes(qt, t_pb):
                    if "tp" in ablate:
                        return
                    for g in range(NKP // 4):
                        pt_ps = ps_t.tile([128, 4, 128], F16, tag="pt_ps",
                                          name=f"pt_ps_{qt}_{g}")
                        for j in range(4):
                            kp = g * 4 + j
                            nc.tensor.transpose(
                                pt_ps[:, j, :],
                                t_pb[:, kp * 128 : (kp + 1) * 128],
                                t_ident[:],
                            )
                        nc.scalar.copy(
                            t_PT[:, g * 4 : (g + 1) * 4, qt * 128 : (qt + 1) * 128],
                            pt_ps[:],
                        )

                for qt in range(NDQ):
                    if qt + 3 < NDQ:
                        load_qt(qt + 3)
                    if qt == 3:
                        # stream V during the back half of phase 1
                        for kp in range(NKP):
                            nc.sync.dma_start(t_v[:, kp, :], v_b3[:, kp, :])
                    t_qT = qt_tiles.pop(qt)

                    s_banks = [
                        ps_s.tile([128, 512], F32, name=f"s_ps_{qt}_{kc}", tag="s_ps")
                        for kc in range(NKC)
                    ]
                    if qt == 0:
                        # kc-outer so matmuls chase the arriving K^T chunks
                        for kc in range(NKC):
                            for dc in range(NDC):
                                nc.tensor.matmul(
                                    s_banks[kc][:],
                                    t_qT[:, dc, :],
                                    t_kT[:, dc, kc * 512 : (kc + 1) * 512],
                                    start=(dc == 0),
                                    stop=(dc == NDC - 1),
                                )
                    else:
                        for dc in range(NDC):
                            for kc in range(NKC):
                                nc.tensor.matmul(
                                    s_banks[kc][:],
                                    t_qT[:, dc, :],
                                    t_kT[:, dc, kc * 512 : (kc + 1) * 512],
                                    start=(dc == 0),
                                    stop=(dc == NDC - 1),
                                )

                    nc.vector.tensor_copy(
                        t_qTb[:, :, qt * 128 : (qt + 1) * 128], t_qT[:]
                    )

                    # software-pipelined: transpose previous q-tile's P
                    if prev_pb is not None:
                        emit_transposes(qt - 1, prev_pb)

                    # evac + mask add (DVE), then row max
                    t_s = p1_work.tile([128, LK], F32, tag="t_s", name=f"t_s_{qt}",
                                       bufs=2)
                    for kc in range(NKC):
                        nc.vector.tensor_tensor(
                            t_s[:, kc * 512 : (kc + 1) * 512],
                            s_banks[kc][:],
                            t_mask[:, kc * 512 : (kc + 1) * 512],
                            op=mybir.AluOpType.add,
                        )
                    if "sm" in ablate:
                        prev_pb = None
                        continue
                    t_negmax = p1_work.tile([128, 1], F32, tag="t_negmax",
                                            name=f"t_negmax_{qt}")
                    nc.vector.reduce_max(
                        t_negmax[:], t_s[:], axis=mybir.AxisListType.X, negate=True
                    )

                    # exp with bias + accumulated row sum (ACT)
                    t_e = p1_work.tile([128, LK], F32, tag="t_e", name=f"t_e_{qt}")
                    t_l = p1_work.tile([128, 1], F32, tag="t_l", name=f"t_l_{qt}")
                    nc.scalar.activation(
                        t_e[:],
                        t_s[:],
                        mybir.ActivationFunctionType.Exp,
                        bias=t_negmax[:],
                        scale=1.0,
                        accum_out=t_l[:],
                    )
                    t_r = p1_work.tile([128, 1], F32, tag="t_r", name=f"t_r_{qt}")
                    nc.vector.reciprocal(t_r[:], t_l[:])

                    # P f32 for attn output (GPSIMD, keeps DVE/ACT free)
                    if "attn" not in ablate:
                        t_p = p1_p.tile([128, LK], F32, tag="t_p", name=f"t_p_{qt}")
                        nc.scalar.activation(
                            t_p[:], t_e[:],
                            mybir.ActivationFunctionType.Copy, scale=t_r[:],
                        )
                        nc.sync.dma_start(attn_o[qt * 128 : (qt + 1) * 128, :], t_p[:])

                    # P bf16 for the PE transpose (DVE)
                    t_pb = p1_pb.tile([128, LK], F16, tag="t_pb", name=f"t_pb_{qt}")
                    nc.vector.tensor_scalar_mul(t_pb[:], t_e[:], t_r[:])
                    prev_pb = t_pb

                if prev_pb is not None:
                    emit_transposes(NDQ - 1, prev_pb)

            if phases == (1,):
                return
            # persistent phases 2-3 (allocated after phase-1 pools release)
            with (
                tc.tile_pool(name="mx_pool", bufs=1) as mx_pool,
                tc.tile_pool(name="p3_w", bufs=1) as p3_w,
            ):
                t_mixT = mx_pool.tile([128, NDC, LQ], F16)
                # W^T + bias stream during phase 2
                t_wT = p3_w.tile([128, NEC, D], F16)
                for ec in range(NEC):
                    nc.sync.dma_start(t_wT[:, ec, :], wT_b3[:, ec, :])
                t_bias = p3_w.tile([128, D], F32)
                nc.sync.dma_start(t_bias[:], bias_b[:])

                # ---------------- Phase 2: mix^T = V^T @ P^T ----------------
                with tc.tile_pool(name="ps_m", bufs=3, space="PSUM") as ps_m:
                    for st in range(LQ // 512):
                        for dc in range(NDC):
                            m_ps = ps_m.tile([128, 512], F32, tag="m_ps",
                                             name=f"m_ps_{st}_{dc}")
                            for kp in range(NKP):
                                nc.tensor.matmul(
                                    m_ps[:],
                                    t_v[:, kp, dc * 128 : (dc + 1) * 128],
                                    t_PT[:, kp, st * 512 : (st + 1) * 512],
                                    start=(kp == 0),
                                    stop=(kp == NKP - 1),
                                )
                            nc.vector.tensor_copy(
                                t_mixT[:, dc, st * 512 : (st + 1) * 512], m_ps[:]
                            )

                if 3 not in phases:
                    return
                # ------------- Phase 3: out = tanh([mix;q] @ W^T + b) -------------
                with (
                    tc.tile_pool(name="p3_o", bufs=4) as p3_o,
                    tc.tile_pool(name="ps_o", bufs=4, space="PSUM") as ps_o,
                ):
                    for qt in range(NDQ):
                        o_banks = [
                            ps_o.tile([128, 512], F32, name=f"o_ps_{qt}_{dh}",
                                      tag="o_ps")
                            for dh in range(2)
                        ]
                        for ec in range(NEC):
                            if ec < NDC:
                                lhsT = t_mixT[:, ec, qt * 128 : (qt + 1) * 128]
                            else:
                                lhsT = t_qTb[:, ec - NDC, qt * 128 : (qt + 1) * 128]
                            for dh in range(2):
                                nc.tensor.matmul(
                                    o_banks[dh][:],
                                    lhsT,
                                    t_wT[:, ec, dh * 512 : (dh + 1) * 512],
                                    start=(ec == 0),
                                    stop=(ec == NEC - 1),
                                )
                        for dh in range(2):
                            t_o = p3_o.tile([128, 512], F32, tag="t_o",
                                            name=f"t_o_{qt}_{dh}")
                            nc.vector.scalar_tensor_tensor(
                                t_o[:],
                                o_banks[dh][:],
                                1.0,
                                t_bias[:, dh * 512 : (dh + 1) * 512],
                                op0=mybir.AluOpType.mult,
                                op1=mybir.AluOpType.add,
                            )
                            t_oo = p3_o.tile([128, 512], F32, tag="t_oo",
                                             name=f"t_oo_{qt}_{dh}")
                            nc.scalar.activation(
                                t_oo[:], t_o[:], mybir.ActivationFunctionType.Tanh
                            )
                            nc.sync.dma_start(
                                out_o[qt * 128 : (qt + 1) * 128,
                                      dh * 512 : (dh + 1) * 512],
                                t_oo[:],
                            )

        if repeat == 1:
            emit_body()
        else:
            with tc.For_i(0, repeat, 1):
                emit_body()

        qtb_pool.release()
        v_pool.release()
        pt_pool.release()
        const_pool.release()

    nc.compile()
    return nc


_NC_CACHE = []


def _get_nc():
    if not _NC_CACHE:
        _NC_CACHE.append(_build_program())
    return _NC_CACHE[0]


def _prep_in_maps(output, context, mask, W, b):
    """Host-side shard + layout prep. One in_map per core (= per batch)."""
    output = np.ascontiguousarray(output, dtype=np.float32)
    context = np.ascontiguousarray(context, dtype=np.float32)
    W = np.ascontiguousarray(W, dtype=np.float32)
    b = np.ascontiguousarray(b, dtype=np.float32)

    wT = np.ascontiguousarray(W.T)                      # [2D, D]
    wT_b16 = wT.astype(np.float16)
    bias_b = np.ascontiguousarray(
        np.broadcast_to(b[None, :], (128, D)), dtype=np.float32
    )

    in_maps = []
    for core in range(N_CORES):
        qT = np.ascontiguousarray(output[core].T)       # [D, LQ]
        kT = np.ascontiguousarray(context[core].T)      # [D, LK]
        mrow = mask[core, 0, :]                         # [LK] bool
        madd = np.where(mrow, NEG_BIG, np.float32(0.0)).astype(ml_dtypes.bfloat16)
        maskb = np.ascontiguousarray(np.broadcast_to(madd[None, :], (128, LK)))
        in_maps.append(
            {
                "qT_r": qT.astype(np.float16),
                "kT_r": kT.astype(np.float16),
                "v_b": context[core].astype(np.float16),
                "wT_b": wT_b16,
                "maskb": maskb,
                "bias_b": bias_b,
            }
        )
    return in_maps


def _mask_is_row_constant(mask):
    return bool(np.all(mask == mask[:, :1, :]))


def _reference_fallback(output, context, mask, W, b):
    """Numpy fallback for mask shapes the device kernel doesn't handle
    (the reference always produces row-constant key-padding masks)."""
    attn = np.einsum("bqd,bkd->bqk", output, context).astype(np.float32)
    attn = np.where(mask, -np.inf, attn)
    m = attn.max(axis=-1, keepdims=True)
    e = np.exp(attn - m)
    attn = e / e.sum(axis=-1, keepdims=True)
    mix = np.einsum("bqk,bkd->bqd", attn, context)
    combined = np.concatenate((mix, output), axis=-1)
    out = np.tanh(np.einsum("bqe,de->bqd", combined, W) + b)
    return out.astype(np.float32), attn.astype(np.float32)


def _run(in_maps, trace=False, nc=None, **kw):
    if nc is None:
        nc = _get_nc()
    return bass_utils.run_bass_kernel_spmd(
        nc, in_maps, core_ids=list(range(N_CORES)), trace=trace, **kw
    )


def kernel(output, context, mask, W, b):
    if not _mask_is_row_constant(mask):
        return _reference_fallback(
            np.asarray(output, dtype=np.float32),
            np.asarray(context, dtype=np.float32),
            np.asarray(mask),
            np.asarray(W, dtype=np.float32),
            np.asarray(b, dtype=np.float32),
        )
    in_maps = _prep_in_maps(output, context, mask, W, b)
    res = _run(in_maps)
    out = np.stack([res.results[c]["out_o"] for c in range(N_CORES)])
    attn = np.stack([res.results[c]["attn_o"] for c in range(N_CORES)])
    return out.astype(np.float32), attn.astype(np.float32)
